# revision 6
# baseline (speedup 1.0000x reference)
"""Trainium2 Bass kernel for nn_BFR3 (gnn_message_passing).

Algebraic collapse of the reference:
  - The [B, G*G, 2H] edge tensor never materializes. gate[b,i,j] =
    sigmoid(u[b,j] + v[b,i] + eb) with u = h @ ew[:H], v = h @ ew[H:].
  - Message aggregation: recv[...,:H] = (gate*mask) @ h (PE matmul),
    recv[...,H:] = h * rowsum(gate*mask).
  - The hypergraph double scatter collapses to dinv * (M.T @ (binv * (M @
    sum_b(upd2 @ hg_w.T)))) with M the [NHE, G] incidence-count matrix;
    the result is identical for every batch.

Sharding: 8 cores each own 150 genes (all batches). BatchNorm (per gene
over batch x feat) is core-local. Three collectives: AllGather of h after
the infer MLP (round 1 needs every source gene), AllGather of h2bn before
round 2, and an AllReduce over the shared hyperedge features.

Dispatch-path optimizations (the measured time is dominated by the PJRT/
axon dispatch, not silicon):
  - jax persistent compilation cache is enabled at import so the warm
    dispatch skips the per-call BIR->NEFF recompile.
  - All inputs are packed into TWO arrays per core (one f32, one u8,
    ~128 KB total vs 26 arrays / 960 KB before): edge masks are
    bit-packed (8x) and unpacked on-device with shift/and; the
    incidence matrix ships as raw u8 counts scaled on-device by
    binv/dinv; the replicated full x is gone (own slice + AllGather).
"""
import os
import sys
import tempfile

import numpy as np

sys.path.insert(0, "/opt/trn_rl_repo")

import jax  # noqa: E402

try:
    _cdir = os.environ.get("JAX_COMPILATION_CACHE_DIR") or os.path.join(
        tempfile.gettempdir(), "jax_cc_cache")
    os.makedirs(_cdir, exist_ok=True)
    jax.config.update("jax_compilation_cache_dir", _cdir)
    jax.config.update("jax_persistent_cache_min_entry_size_bytes", -1)
    jax.config.update("jax_persistent_cache_min_compile_time_secs", 0.0)
except Exception:
    pass

import concourse.bass as bass  # noqa: E402,F401
import concourse.bacc as bacc  # noqa: E402
import concourse.mybir as mybir  # noqa: E402
import concourse.tile as tile  # noqa: E402
from concourse import bass_utils  # noqa: E402

B, G, NIN, H = 4, 1200, 10, 4
NHE, NINC = 300, 4800
ALPHA, BETA = 0.005, 5e-5
BN_EPS = 1e-5
NCORES = 8
SL = G // NCORES            # 150 genes per core
BI = B * SL                 # 600 (b,i) pairs per core
JT = 120                    # j-tile partition size
NJ = G // JT                # 10 j-tiles per batch
NT = B * NJ                 # 40 (b,j) tiles
MB = 19                     # packed mask bytes per row per round
F32 = mybir.dt.float32
U8 = mybir.dt.uint8
AF = mybir.ActivationFunctionType
OP = mybir.AluOpType
AX = mybir.AxisListType

_COMPILED = {}

# ---- packed-f32 input layout (offsets in floats) ----
_FOFF = {}
_fcur = 0
for _nm, _n in [
    ("xTaIc", (NIN + 1) * BI), ("wE", (NIN + 1) * 5),
    ("ewhi1", 5), ("ewhi2", 5),
    ("nwE1a", 20), ("nwE1b", 16), ("mwE1a", 20), ("mwE1b", 16),
    ("nwE2a", 20), ("nwE2b", 16), ("mwE2a", 20), ("mwE2b", 16),
    ("mm3Ea", 20), ("mm3Eb", 16), ("hgwT", 16), ("hgb", 4),
    ("ewlo1", NT * 5), ("ewlo2", NT * 5),
    ("w1r", BI), ("b1r", BI), ("binv", NHE), ("dinv", SL),
]:
    _FOFF[_nm] = _fcur
    _fcur += _n
FPK_N = _fcur

# ---- packed-u8 input layout (offsets in bytes) ----
# incidence counts are <= 15, so they ship nibble-packed (two per byte)
UOFF_MASKS = 0                       # [G, 2*MB] bit-packed masks
UOFF_MT = G * 2 * MB                 # [SL, NHE/2] M[:,Ic].T nibbles
UOFF_MN = UOFF_MT + SL * NHE // 2    # [NHE, SL/2] M[:,Ic] nibbles
UPK_N = UOFF_MN + NHE * SL // 2


def _elu(nc, pool, out_ap, in_ap, shape):
    tmin = pool.tile(list(shape), F32, tag="elu_min", name="elu_min", bufs=4)
    texp = pool.tile(list(shape), F32, tag="elu_exp", name="elu_exp", bufs=4)
    nc.vector.tensor_scalar_min(tmin[:], in_ap, 0.0)
    nc.scalar.activation(texp[:], tmin[:], AF.Exp)
    nc.vector.scalar_tensor_tensor(out_ap, texp[:], -1.0, in_ap, OP.add, OP.max)


def _build():
    nc = bacc.Bacc("TRN2", target_bir_lowering=False, debug=False,
                   num_devices=NCORES)
    fpk = nc.dram_tensor("fpk", [1, FPK_N], F32, kind="ExternalInput")
    upk = nc.dram_tensor("upk", [1, UPK_N], U8, kind="ExternalInput")
    out_d = nc.dram_tensor("outT", [4, BI], F32, kind="ExternalOutput")

    def fap(name, a, b):
        n = a * b
        o = _FOFF[name]
        return fpk[:, o:o + n].rearrange("z (a b) -> (z a) b", a=a, b=b)

    def frow(name, n):
        o = _FOFF[name]
        return fpk[:, o:o + n]

    with tile.TileContext(nc) as tc:
        with (
            tc.tile_pool(name="p", bufs=1) as p,        # persistent
            tc.tile_pool(name="w", bufs=1) as w,        # rotating scratch
            tc.tile_pool(name="psA", bufs=3, space="PSUM") as psA,
            tc.tile_pool(name="dram", bufs=1, space="DRAM") as dr,
        ):
            # ---- unpack the f32 pack into SBUF tiles ----
            sb = {}
            for name, a, b in [
                ("xTaIc", NIN + 1, BI), ("wE", NIN + 1, 5),
                ("ewhi1", 5, 1), ("ewhi2", 5, 1),
                ("nwE1a", 5, 4), ("nwE1b", 4, 4), ("mwE1a", 5, 4),
                ("mwE1b", 4, 4), ("nwE2a", 5, 4), ("nwE2b", 4, 4),
                ("mwE2a", 5, 4), ("mwE2b", 4, 4), ("mm3Ea", 5, 4),
                ("mm3Eb", 4, 4), ("hgwT", 4, 4), ("hgb", 4, 1),
            ]:
                sb[name] = p.tile([a, b], F32, tag=name, name=f"sb_{name}")
                nc.sync.dma_start(sb[name][:], fap(name, a, b))
            for name, n in [("ewlo1", NT * 5), ("ewlo2", NT * 5),
                            ("w1r", BI), ("b1r", BI), ("dinv", SL)]:
                sb[name] = p.tile([1, n], F32, tag=name, name=f"sb_{name}")
                nc.sync.dma_start(sb[name][:], frow(name, n))
            # binv in the e = k*100 + p layout used by e_nat
            binv_col = p.tile([100, 3], F32, tag="binv_col")
            nc.sync.dma_start(
                binv_col[:],
                frow("binv", NHE).rearrange("z (k q) -> (z q) k", k=3, q=100))

            # ---- bit-packed masks -> m_sb[r] [JT, NJ, SL] f32 ----
            mraw = p.tile([JT, NJ, 2 * MB], U8, tag="mraw")
            nc.sync.dma_start(
                mraw[:],
                upk[:, UOFF_MASKS:UOFF_MASKS + G * 2 * MB].rearrange(
                    "z (jt q c) -> (z q) jt c", jt=NJ, q=JT, c=2 * MB))
            m_sb = {}
            for r, coef in ((1, ALPHA), (2, BETA)):
                off = (r - 1) * MB
                bits = w.tile([JT, NJ, 8 * MB], U8, tag=f"m{r}bits",
                              name=f"m{r}bits")
                bview = bits[:].rearrange("p t (c k) -> p t c k", k=8)
                for k in range(8):
                    nc.vector.tensor_scalar(
                        bview[:, :, :, k], mraw[:, :, off:off + MB],
                        k, 1, OP.logical_shift_right, OP.bitwise_and)
                t = p.tile([JT, NJ, SL], F32, tag=f"m{r}sb", name=f"m{r}sb")
                nc.vector.tensor_scalar(
                    t[:], bits[:, :, 0:SL],
                    1.0 - coef, coef, OP.mult, OP.add)
                m_sb[r] = t

            # ---- nibble-packed incidence counts -> f32 tiles ----
            def unpack_nibbles(tag, off, kdim, qdim, cdim):
                pk = w.tile([qdim, kdim, cdim], U8, tag=f"{tag}pk",
                            name=f"{tag}pk")
                nc.sync.dma_start(
                    pk[:],
                    upk[:, off:off + kdim * qdim * cdim].rearrange(
                        "z (k q c) -> (z q) k c", k=kdim, q=qdim, c=cdim))
                bits = w.tile([qdim, kdim, 2 * cdim], U8, tag=f"{tag}bits",
                              name=f"{tag}bits")
                bview = bits[:].rearrange("p k (c j) -> p k c j", j=2)
                for j in range(2):
                    nc.vector.tensor_scalar(
                        bview[:, :, :, j], pk[:],
                        4 * j, 15, OP.logical_shift_right, OP.bitwise_and)
                t = p.tile([qdim, kdim, 2 * cdim], F32, tag=tag, name=tag)
                nc.vector.tensor_scalar_mul(
                    t[:].rearrange("p k e -> p (k e)"),
                    bits[:].rearrange("p k e -> p (k e)"), 1.0)
                return t

            mt_sb = unpack_nibbles("mt_sb", UOFF_MT, 2, 75, NHE // 2)
            mn_sb = unpack_nibbles("mn_sb", UOFF_MN, 3, 100, SL // 2)

            ones4 = p.tile([4, 1], F32, tag="ones4")
            nc.vector.memset(ones4[:], 1.0)

            ewlo_bc = {}
            for r, nm in ((1, "ewlo1"), (2, "ewlo2")):
                t = p.tile([JT, NT * 5], F32, tag=f"ewlo{r}bc",
                           name=f"ewlo{r}bc")
                nc.gpsimd.partition_broadcast(t[:], sb[nm][:])
                ewlo_bc[r] = t

            # ---- h = elu(x @ infer_w.T + infer_b), own slice only ----
            hTIc1 = p.tile([5, BI], F32, tag="hTIc1")
            for half in range(2):
                cs = slice(half * 300, half * 300 + 300)
                hp = psA.tile([5, 300], F32, tag="psA_gen", name="hps")
                nc.tensor.matmul(hp[:], sb["wE"][:], sb["xTaIc"][:, cs],
                                 start=True, stop=True)
                _elu(nc, w, hTIc1[:, cs], hp[:], (5, 300))

            def allgather_hN(src44, tag):
                """AllGather own [4, BI] slice -> hN [JT, NT, 5] (full h)."""
                agin = dr.tile([BI, 4], F32)
                agout = dr.tile([NCORES * BI, 4], F32, addr_space="Shared")
                nc.sync.dma_start(agin[:].rearrange("bi f -> f bi"), src44)
                nc.gpsimd.collective_compute(
                    "AllGather", OP.bypass,
                    replica_groups=[list(range(NCORES))],
                    ins=[agin[:].opt()], outs=[agout[:].opt()])
                hN = p.tile([JT, NT, 5], F32, tag=tag, name=tag)
                for cp in range(NCORES):
                    j0 = cp * SL
                    jt0, p0 = j0 // JT, j0 % JT
                    len0 = min(SL, JT - p0)
                    runs = [(jt0, p0, 0, len0)]
                    if len0 < SL:
                        runs.append((jt0 + 1, 0, len0, SL - len0))
                    for (jt, pstart, i0, ln) in runs:
                        dst = hN[pstart:pstart + ln, :, 0:4] \
                            .rearrange("p (b jt) f -> p b jt f", b=B)[:, :, jt, :]
                        sap = agout[cp * BI:(cp + 1) * BI, :] \
                            .rearrange("(b i) f -> i b f", b=B)[i0:i0 + ln, :, :]
                        nc.sync.dma_start(dst, sap)
                nc.vector.memset(hN[:, :, 4:5], 1.0)
                return hN

            hN1 = allgather_hN(hTIc1[0:4, :], "hN1")

            def bn(yT, tag):
                """BatchNorm per gene over (batch, feat); yT [4, BI] sbuf AP.
                Two-pass: mean, subtract, then variance of the residual."""
                srow = w.tile([1, BI], F32, tag="bn_sr", name="bn_sr")
                for half in range(2):
                    cs = slice(half * 300, half * 300 + 300)
                    sp = psA.tile([1, 300], F32, tag="psA_gen", name="bn_sp")
                    nc.tensor.matmul(sp[:], ones4[:], yT[:, cs],
                                     start=True, stop=True)
                    nc.vector.tensor_copy(srow[:, cs], sp[:])
                m = w.tile([1, SL], F32, tag="bn_m", name="bn_m")
                nc.vector.tensor_reduce(
                    m[:], srow[:].rearrange("p (b i) -> p i b", b=B),
                    AX.X, OP.add)
                nc.vector.tensor_scalar_mul(m[:], m[:], 1.0 / 16.0)
                m600 = w.tile([1, BI], F32, tag="bn_m600", name="bn_m600")
                for b in range(B):
                    cs = slice(b * SL, b * SL + SL)
                    nc.vector.tensor_copy(m600[:, cs], m[:])
                mbc = w.tile([4, BI], F32, tag="bn_mbc", name="bn_mbc")
                nc.gpsimd.partition_broadcast(mbc[:], m600[:])
                ybar = w.tile([4, BI], F32, tag="bn_ybar", name="bn_ybar")
                nc.vector.tensor_sub(ybar[:], yT, mbc[:])
                sq = w.tile([4, BI], F32, tag="bn_sq", name="bn_sq")
                nc.vector.tensor_tensor(sq[:], ybar[:], ybar[:], OP.mult)
                qrow = w.tile([1, BI], F32, tag="bn_qr", name="bn_qr")
                for half in range(2):
                    cs = slice(half * 300, half * 300 + 300)
                    qp = psA.tile([1, 300], F32, tag="psA_gen", name="bn_qp")
                    nc.tensor.matmul(qp[:], ones4[:], sq[:, cs],
                                     start=True, stop=True)
                    nc.vector.tensor_copy(qrow[:, cs], qp[:])
                var = w.tile([1, SL], F32, tag="bn_var", name="bn_var")
                nc.vector.tensor_reduce(
                    var[:], qrow[:].rearrange("p (b i) -> p i b", b=B),
                    AX.X, OP.add)
                nc.vector.tensor_scalar(var[:], var[:], 1.0 / 16.0, BN_EPS,
                                        OP.mult, OP.add)
                rec = w.tile([1, SL], F32, tag="bn_rec", name="bn_rec")
                nc.vector.reciprocal(rec[:], var[:])
                rstd = w.tile([1, SL], F32, tag="bn_rstd", name="bn_rstd")
                nc.scalar.activation(rstd[:], rec[:], AF.Sqrt)
                r600 = w.tile([1, BI], F32, tag="bn_r600", name="bn_r600")
                for b in range(B):
                    cs = slice(b * SL, b * SL + SL)
                    nc.vector.tensor_copy(r600[:, cs], rstd[:])
                rbc = w.tile([4, BI], F32, tag="bn_rbc", name="bn_rbc")
                nc.gpsimd.partition_broadcast(rbc[:], r600[:])
                out = p.tile([4, BI], F32, tag=f"{tag}out", name=f"{tag}out")
                nc.vector.tensor_tensor(out[:], ybar[:], rbc[:], OP.mult)
                return out

            def round_(r, hN, hT_ic, ewhi, nwEa, nwEb, mwEa, mwEb):
                """One round. hN [120,40,5]; hT_ic [5,BI] (row 4 ones).
                Returns updT [4, BI]."""
                vrow = w.tile([1, BI], F32, tag="rnd_vrow", name="rnd_vrow")
                for half in range(2):
                    cs = slice(half * 300, half * 300 + 300)
                    vp = psA.tile([1, 300], F32, tag="psA_gen", name="vp")
                    nc.tensor.matmul(vp[:], ewhi, hT_ic[:, cs],
                                     start=True, stop=True)
                    nc.vector.tensor_copy(vrow[:, cs], vp[:])
                vb = p.tile([128, BI], F32, tag="vb", name="vb")
                nc.gpsimd.partition_broadcast(vb[:], vrow[:])
                scr = w.tile([JT, NT * 5], F32, tag="uscr")
                nc.vector.tensor_tensor(
                    scr[:], hN[:].rearrange("p t f -> p (t f)"),
                    ewlo_bc[r][:], OP.mult)
                ucol = w.tile([JT, NT], F32, tag="rnd_ucol", name="rnd_ucol")
                nc.vector.tensor_reduce(
                    ucol[:], scr[:].rearrange("p (t f) -> p t f", f=5),
                    AX.X, OP.add)
                recv1o = w.tile([5, BI], F32, tag="rnd_recv1", name="rnd_recv1")
                nc.vector.memset(recv1o[:, :], 1.0)
                rsrow = w.tile([1, BI], F32, tag="rnd_rs", name="rnd_rs")
                for b in range(B):
                    Wb = w.tile([JT, NJ, SL], F32, tag="Wb", name="Wb", bufs=3)
                    for jt in range(NJ):
                        t = b * NJ + jt
                        nc.scalar.activation(
                            Wb[:, jt, :], vb[0:JT, b * SL:(b + 1) * SL],
                            AF.Sigmoid, bias=ucol[:, t:t + 1])
                    eng = nc.vector if b % 2 == 0 else nc.gpsimd
                    eng.tensor_tensor(
                        Wb[:].rearrange("p t i -> p (t i)"),
                        Wb[:].rearrange("p t i -> p (t i)"),
                        m_sb[r][:].rearrange("p t i -> p (t i)"), OP.mult)
                    rp = psA.tile([5, SL], F32, tag="recvps", name="rp", bufs=2)
                    for jt in range(NJ):
                        t = b * NJ + jt
                        nc.tensor.matmul(rp[:], hN[:, t, :], Wb[:, jt, :],
                                         start=(jt == 0), stop=(jt == NJ - 1))
                    cs = slice(b * SL, (b + 1) * SL)
                    nc.vector.tensor_copy(recv1o[0:4, cs], rp[0:4, :])
                    # rs row: DMA (not a compute op) — partition-offset APs are
                    # only broken on compute engines
                    rv5 = w.tile([5, SL], F32, tag="rv5", name="rv5", bufs=2)
                    nc.vector.tensor_copy(rv5[:], rp[:])
                    nc.sync.dma_start(rsrow[:, cs], rv5[4:5, :])
                rsbc = w.tile([4, BI], F32, tag="rnd_rsbc", name="rnd_rsbc")
                nc.gpsimd.partition_broadcast(rsbc[:], rsrow[:])
                recv2 = w.tile([4, BI], F32, tag="rnd_recv2", name="rnd_recv2")
                nc.vector.tensor_tensor(recv2[:], hT_ic[0:4, :], rsbc[:],
                                        OP.mult)
                # A = elu(nwA @ [recv1;1] + nwB @ recv2); Acat row 4 stays ones
                Acat = w.tile([5, BI], F32, tag="rnd_Acat", name="rnd_Acat")
                nc.vector.memset(Acat[:, :], 1.0)
                for half in range(2):
                    cs = slice(half * 300, half * 300 + 300)
                    ap = psA.tile([4, 300], F32, tag="psA_gen", name="ap")
                    nc.tensor.matmul(ap[:], nwEa, recv1o[:, cs],
                                     start=True, stop=False)
                    nc.tensor.matmul(ap[:], nwEb, recv2[:, cs],
                                     start=False, stop=True)
                    _elu(nc, w, Acat[0:4, cs], ap[:], (4, 300))
                updT = p.tile([4, BI], F32, tag=f"r{r}upd")
                for half in range(2):
                    cs = slice(half * 300, half * 300 + 300)
                    up = psA.tile([4, 300], F32, tag="psA_gen", name="up")
                    nc.tensor.matmul(up[:], mwEa, Acat[:, cs],
                                     start=True, stop=False)
                    nc.tensor.matmul(up[:], mwEb, hT_ic[0:4, cs],
                                     start=False, stop=True)
                    _elu(nc, w, updT[:, cs], up[:], (4, 300))
                return updT

            # ================= round 1 =================
            upd1 = round_(1, hN1, hTIc1[:], sb["ewhi1"][:], sb["nwE1a"][:],
                          sb["nwE1b"][:], sb["mwE1a"][:], sb["mwE1b"][:])
            # h2 = elu(upd1 * diag(W1) + b1), then BN
            w1bc = w.tile([4, BI], F32, tag="w1bc")
            b1bc = w.tile([4, BI], F32, tag="b1bc")
            nc.gpsimd.partition_broadcast(w1bc[:], sb["w1r"][:])
            nc.gpsimd.partition_broadcast(b1bc[:], sb["b1r"][:])
            h2pre = w.tile([4, BI], F32, tag="h2pre")
            nc.vector.tensor_tensor(h2pre[:], upd1[:], w1bc[:], OP.mult)
            nc.vector.tensor_add(h2pre[:], h2pre[:], b1bc[:])
            h2T = w.tile([4, BI], F32, tag="h2T")
            _elu(nc, w, h2T[:], h2pre[:], (4, BI))
            h2bn = bn(h2T[:], "bn1")

            # ---- AllGather #2: h2bn slices -> full h in hN2 layout ----
            hN2 = allgather_hN(h2bn[:], "hN2")
            hTIc2 = p.tile([5, BI], F32, tag="hTIc2")
            nc.vector.memset(hTIc2[:, :], 1.0)
            nc.vector.tensor_copy(hTIc2[0:4, :], h2bn[:])

            # ================= round 2 =================
            upd2 = round_(2, hN2, hTIc2[:], sb["ewhi2"][:], sb["nwE2a"][:],
                          sb["nwE2b"][:], sb["mwE2a"][:], sb["mwE2b"][:])
            upd2bn = bn(upd2[:], "bn2")

            # ---- hypergraph partial: E_part = M[:,Ic] @ (sum_b upd2bn @ hg_w.T)
            s0T = w.tile([4, SL], F32, tag="s0T")
            nc.vector.tensor_reduce(
                s0T[:], upd2bn[:].rearrange("p (b i) -> p i b", b=B),
                AX.X, OP.add)
            s1p = psA.tile([4, SL], F32, tag="psA_gen", name="s1p")
            nc.tensor.matmul(s1p[:], sb["hgwT"][:], s0T[:], start=True,
                             stop=True)
            s1sb = w.tile([4, SL], F32, tag="s1sb")
            nc.vector.tensor_copy(s1sb[:], s1p[:])
            s1d = dr.tile([SL, 4], F32)
            nc.sync.dma_start(s1d[:].rearrange("i f -> f i"), s1sb[:])
            s1n = p.tile([75, 2, 4], F32, tag="s1n")
            nc.sync.dma_start(
                s1n[:], s1d[:].rearrange("(k q) f -> q k f", q=75))
            ep = psA.tile([4, NHE], F32, tag="psA_gen", name="ep")
            for k in range(2):
                nc.tensor.matmul(ep[:], s1n[:, k, :], mt_sb[:, k, :],
                                 start=(k == 0), stop=(k == 1))

            # ---- AllReduce: E = sum over cores of E_part (natural [NHE,4]) ----
            epsb = w.tile([4, NHE], F32, tag="epsb")
            nc.vector.tensor_copy(epsb[:], ep[:])
            arin = dr.tile([NHE, 4], F32)
            arout = dr.tile([NHE, 4], F32, addr_space="Shared")
            nc.sync.dma_start(arin[:].rearrange("e f -> f e"), epsb[:])
            nc.gpsimd.collective_compute(
                "AllReduce", OP.add,
                replica_groups=[list(range(NCORES))],
                ins=[arin[:].opt()], outs=[arout[:].opt()])
            e_nat = p.tile([100, 3, 4], F32, tag="e_nat")
            nc.sync.dma_start(
                e_nat[:], arout[:].rearrange("(k q) f -> q k f", q=100))
            # scale hyperedge features by binv (e = k*100 + p layout)
            for k in range(3):
                nc.vector.tensor_scalar_mul(
                    e_nat[:, k, :], e_nat[:, k, :], binv_col[:, k:k + 1])
            hxp = psA.tile([4, SL], F32, tag="psA_gen", name="hxp")
            for k in range(3):
                nc.tensor.matmul(hxp[:], e_nat[:, k, :], mn_sb[:, k, :],
                                 start=(k == 0), stop=(k == 2))
            # scale node features by dinv (free-dim vector), add bias, elu
            dbc = w.tile([4, SL], F32, tag="dbc")
            nc.gpsimd.partition_broadcast(dbc[:], sb["dinv"][:])
            hxs = w.tile([4, SL], F32, tag="hxs")
            nc.vector.tensor_tensor(hxs[:], hxp[:], dbc[:], OP.mult)
            hxpre = w.tile([4, SL], F32, tag="hxpre")
            nc.vector.tensor_scalar_add(hxpre[:], hxs[:], sb["hgb"][:])
            hxT = w.tile([4, SL], F32, tag="hxT")
            _elu(nc, w, hxT[:], hxpre[:], (4, SL))

            # ---- final: out = elu(mm3A @ [upd2bn;1] + mm3B @ hx + b) ----
            u2cat = w.tile([5, BI], F32, tag="u2cat")
            nc.vector.memset(u2cat[:, :], 1.0)
            nc.vector.tensor_copy(u2cat[0:4, :], upd2bn[:])
            hx600 = w.tile([4, BI], F32, tag="hx600")
            for b in range(B):
                cs = slice(b * SL, (b + 1) * SL)
                nc.vector.tensor_copy(hx600[:, cs], hxT[:])
            outT = w.tile([4, BI], F32, tag="outTsb")
            for half in range(2):
                cs = slice(half * 300, half * 300 + 300)
                op_ = psA.tile([4, 300], F32, tag="psA_gen", name="op_")
                nc.tensor.matmul(op_[:], sb["mm3Ea"][:], u2cat[:, cs],
                                 start=True, stop=False)
                nc.tensor.matmul(op_[:], sb["mm3Eb"][:], hx600[:, cs],
                                 start=False, stop=True)
                _elu(nc, w, outT[:, cs], op_[:], (4, 300))
            nc.sync.dma_start(out_d[:], outT[:])

    nc.compile()
    return nc


def _prep_inputs(x, edge1, edge2, W1, b1, infer_w, infer_b, mlp_e1_w, mlp_e1_b,
                 mlp_e2_w, mlp_e2_b, nodes1_w, nodes1_b, nodes2_w, nodes2_b,
                 mm1_w, mm1_b, mm2_w, mm2_b, mm3_w, mm3_b, hg_w, hg_b,
                 hyper_nodes, hyper_edges):
    f = np.float32
    xT = np.ascontiguousarray(x.transpose(0, 2, 1).astype(f))  # [B, NIN, G]
    xTa = np.concatenate([xT.transpose(1, 0, 2).reshape(NIN, B * G),
                          np.ones((1, B * G), f)], axis=0)
    wE = np.zeros((NIN + 1, 5), f)
    wE[:NIN, :4] = infer_w.T
    wE[NIN, :4] = infer_b
    wE[NIN, 4] = 1.0

    def split5(wgt, bias):
        a = np.zeros((5, 4), f)
        a[:4] = wgt[:, :4].T
        a[4] = bias
        b_ = np.ascontiguousarray(wgt[:, 4:].T.astype(f))
        return a, b_

    nwE1a, nwE1b = split5(nodes1_w, nodes1_b)
    mwE1a, mwE1b = split5(mm1_w, mm1_b)
    nwE2a, nwE2b = split5(nodes2_w, nodes2_b)
    mwE2a, mwE2b = split5(mm2_w, mm2_b)
    mm3Ea, mm3Eb = split5(mm3_w, mm3_b)

    def ewparts(ew, eb):
        lo5 = np.zeros(5, f)
        lo5[:4] = ew[0, :4]
        lor = np.tile(lo5, NT)                          # [200]
        hi = np.zeros(5, f)
        hi[:4] = ew[0, 4:8]
        hi[4] = eb[0]
        return lor.astype(f), hi
    ewlo1, ewhi1 = ewparts(mlp_e1_w, mlp_e1_b)
    ewlo2, ewhi2 = ewparts(mlp_e2_w, mlp_e2_b)

    # bit-packed masks: byte c bit k of row j <- edge[j, Ic][8c+k]
    m1 = edge1.T.astype(np.uint8)                       # [G(j), G(i)]
    m2 = edge2.T.astype(np.uint8)

    M = np.zeros((NHE, G), f)
    np.add.at(M, (hyper_edges, hyper_nodes), 1.0)
    deg = M.sum(0)
    dinv = np.where(deg > 0, 1.0 / np.maximum(deg, 1), 0.0).astype(f)
    bdeg = B * M.sum(1)
    binv = np.where(bdeg > 0, 1.0 / np.maximum(bdeg, 1), 0.0).astype(f)
    Mu8 = M.astype(np.uint8)

    w1d = np.diag(W1).astype(f)
    hgwT = hg_w.T.astype(f)
    hgb = hg_b.astype(f)

    in_maps = []
    for c in range(NCORES):
        Ic = slice(c * SL, (c + 1) * SL)
        xTaIc = np.concatenate([xTa[:, b * G + c * SL: b * G + (c + 1) * SL]
                                for b in range(B)], axis=1)
        fpk = np.zeros(FPK_N, f)
        for nm, arr in [
            ("xTaIc", xTaIc), ("wE", wE), ("ewhi1", ewhi1), ("ewhi2", ewhi2),
            ("nwE1a", nwE1a), ("nwE1b", nwE1b), ("mwE1a", mwE1a),
            ("mwE1b", mwE1b), ("nwE2a", nwE2a), ("nwE2b", nwE2b),
            ("mwE2a", mwE2a), ("mwE2b", mwE2b), ("mm3Ea", mm3Ea),
            ("mm3Eb", mm3Eb), ("hgwT", hgwT), ("hgb", hgb),
            ("ewlo1", ewlo1), ("ewlo2", ewlo2),
            ("w1r", np.tile(w1d[Ic], B)), ("b1r", np.tile(b1.astype(f)[Ic], B)),
            ("binv", binv), ("dinv", dinv[Ic]),
        ]:
            av = np.asarray(arr, f).ravel()
            fpk[_FOFF[nm]:_FOFF[nm] + av.size] = av
        upkb = np.zeros(UPK_N, np.uint8)
        for r, msk in ((0, m1), (1, m2)):
            pk = np.packbits(msk[:, Ic], axis=1, bitorder="little")  # [G, 19]
            dst = upkb[UOFF_MASKS:UOFF_MASKS + G * 2 * MB].reshape(G, 2 * MB)
            dst[:, r * MB:r * MB + pk.shape[1]] = pk
        mt = np.ascontiguousarray(Mu8[:, Ic].T)                # [SL, NHE]
        mn = np.ascontiguousarray(Mu8[:, Ic])                  # [NHE, SL]
        upkb[UOFF_MT:UOFF_MT + SL * NHE // 2] = \
            (mt[:, 0::2] | (mt[:, 1::2] << 4)).ravel()
        upkb[UOFF_MN:UOFF_MN + NHE * SL // 2] = \
            (mn[:, 0::2] | (mn[:, 1::2] << 4)).ravel()
        in_maps.append({"fpk": fpk[None, :], "upk": upkb[None, :]})
    return in_maps


def kernel(**inputs):
    inputs = {k: np.asarray(v) for k, v in inputs.items()}
    if "nc" not in _COMPILED:
        _COMPILED["nc"] = _build()
    nc = _COMPILED["nc"]
    in_maps = _prep_inputs(**inputs)
    res = bass_utils.run_bass_kernel_spmd(nc, in_maps,
                                          core_ids=list(range(NCORES)))
    out = np.empty((B, G, H), np.float32)
    for c in range(NCORES):
        oT = res.results[c]["outT"].reshape(4, B, SL)   # [f, b, i]
        out[:, c * SL:(c + 1) * SL, :] = oT.transpose(1, 2, 0)
    return out


# revision 7
# speedup vs baseline: 1.1405x; 1.1405x over previous
"""Trainium2 Bass kernel for nn_BFR3 (gnn_message_passing).

Algebraic collapse of the reference:
  - The [B, G*G, 2H] edge tensor never materializes. gate[b,i,j] =
    sigmoid(u[b,j] + v[b,i] + eb) with u = h @ ew[:H], v = h @ ew[H:].
  - Message aggregation: recv[...,:H] = (gate*mask) @ h (PE matmul),
    recv[...,H:] = h * rowsum(gate*mask).
  - The hypergraph double scatter collapses to dinv * (M.T @ (binv * (M @
    sum_b(upd2 @ hg_w.T)))) with M the [NHE, G] incidence-count matrix;
    the result is identical for every batch.

Sharding: 8 cores each own 150 genes (all batches). BatchNorm (per gene
over batch x feat) is core-local. Three collectives: AllGather of h after
the infer MLP (round 1 needs every source gene), AllGather of h2bn before
round 2, and an AllReduce over the shared hyperedge features.

Dispatch-path optimizations (the measured time is dominated by the PJRT/
axon dispatch, not silicon):
  - jax persistent compilation cache is enabled at import so the warm
    dispatch skips the per-call BIR->NEFF recompile.
  - All inputs are packed into TWO arrays per core (one f32, one u8,
    ~128 KB total vs 26 arrays / 960 KB before): edge masks are
    bit-packed (8x) and unpacked on-device with shift/and; the
    incidence matrix ships as raw u8 counts scaled on-device by
    binv/dinv; the replicated full x is gone (own slice + AllGather).
"""
import os
import sys
import tempfile

import numpy as np

sys.path.insert(0, "/opt/trn_rl_repo")

import jax  # noqa: E402

try:
    _cdir = os.environ.get("JAX_COMPILATION_CACHE_DIR") or os.path.join(
        tempfile.gettempdir(), "jax_cc_cache")
    os.makedirs(_cdir, exist_ok=True)
    jax.config.update("jax_compilation_cache_dir", _cdir)
    jax.config.update("jax_persistent_cache_min_entry_size_bytes", -1)
    jax.config.update("jax_persistent_cache_min_compile_time_secs", 0.0)
except Exception:
    pass

import concourse.bass as bass  # noqa: E402,F401
import concourse.bacc as bacc  # noqa: E402
import concourse.mybir as mybir  # noqa: E402
import concourse.tile as tile  # noqa: E402
from concourse import bass_utils  # noqa: E402

B, G, NIN, H = 4, 1200, 10, 4
NHE, NINC = 300, 4800
ALPHA, BETA = 0.005, 5e-5
BN_EPS = 1e-5
NCORES = 8
SL = G // NCORES            # 150 genes per core
BI = B * SL                 # 600 (b,i) pairs per core
JT = 120                    # j-tile partition size
NJ = G // JT                # 10 j-tiles per batch
NT = B * NJ                 # 40 (b,j) tiles
MB = 19                     # packed mask bytes per row per round
F32 = mybir.dt.float32
U8 = mybir.dt.uint8
AF = mybir.ActivationFunctionType
OP = mybir.AluOpType
AX = mybir.AxisListType

_COMPILED = {}

# ---- packed-f32 input layout (offsets in floats) ----
_FOFF = {}
_fcur = 0
for _nm, _n in [
    ("xTaIc", (NIN + 1) * BI), ("wE", (NIN + 1) * 5),
    ("ewhi1", 5), ("ewhi2", 5),
    ("nwE1a", 20), ("nwE1b", 16), ("mwE1a", 20), ("mwE1b", 16),
    ("nwE2a", 20), ("nwE2b", 16), ("mwE2a", 20), ("mwE2b", 16),
    ("mm3Ea", 20), ("mm3Eb", 16), ("hgwT", 16), ("hgb", 4),
    ("ewlo1", NT * 5), ("ewlo2", NT * 5),
    ("w1r", BI), ("b1r", BI), ("binv", NHE), ("dinv", SL),
]:
    _FOFF[_nm] = _fcur
    _fcur += _n
FPK_N = _fcur

# ---- packed-u8 input layout (offsets in bytes) ----
# incidence counts are <= 15, so they ship nibble-packed (two per byte)
UOFF_MASKS = 0                       # [G, 2*MB] bit-packed masks
UOFF_MT = G * 2 * MB                 # [SL, NHE/2] M[:,Ic].T nibbles
UOFF_MN = UOFF_MT + SL * NHE // 2    # [NHE, SL/2] M[:,Ic] nibbles
UPK_N = UOFF_MN + NHE * SL // 2


def _elu(nc, pool, out_ap, in_ap, shape):
    tmin = pool.tile(list(shape), F32, tag="elu_min", name="elu_min", bufs=4)
    texp = pool.tile(list(shape), F32, tag="elu_exp", name="elu_exp", bufs=4)
    nc.vector.tensor_scalar_min(tmin[:], in_ap, 0.0)
    nc.scalar.activation(texp[:], tmin[:], AF.Exp)
    nc.vector.scalar_tensor_tensor(out_ap, texp[:], -1.0, in_ap, OP.add, OP.max)


def _build():
    nc = bacc.Bacc("TRN2", target_bir_lowering=False, debug=False,
                   num_devices=NCORES)
    fpk = nc.dram_tensor("fpk", [1, FPK_N], F32, kind="ExternalInput")
    upk = nc.dram_tensor("upk", [1, UPK_N], U8, kind="ExternalInput")
    out_d = nc.dram_tensor("outT", [4, BI], F32, kind="ExternalOutput")

    def fap(name, a, b):
        n = a * b
        o = _FOFF[name]
        return fpk[:, o:o + n].rearrange("z (a b) -> (z a) b", a=a, b=b)

    def frow(name, n):
        o = _FOFF[name]
        return fpk[:, o:o + n]

    with tile.TileContext(nc) as tc:
        with (
            tc.tile_pool(name="p", bufs=1) as p,        # persistent
            tc.tile_pool(name="w", bufs=1) as w,        # rotating scratch
            tc.tile_pool(name="psA", bufs=3, space="PSUM") as psA,
            tc.tile_pool(name="dram", bufs=1, space="DRAM") as dr,
        ):
            # ---- unpack the f32 pack into SBUF tiles ----
            sb = {}
            for name, a, b in [
                ("xTaIc", NIN + 1, BI), ("wE", NIN + 1, 5),
                ("ewhi1", 5, 1), ("ewhi2", 5, 1),
                ("nwE1a", 5, 4), ("nwE1b", 4, 4), ("mwE1a", 5, 4),
                ("mwE1b", 4, 4), ("nwE2a", 5, 4), ("nwE2b", 4, 4),
                ("mwE2a", 5, 4), ("mwE2b", 4, 4), ("mm3Ea", 5, 4),
                ("mm3Eb", 4, 4), ("hgwT", 4, 4), ("hgb", 4, 1),
            ]:
                sb[name] = p.tile([a, b], F32, tag=name, name=f"sb_{name}")
                nc.sync.dma_start(sb[name][:], fap(name, a, b))
            for name, n in [("ewlo1", NT * 5), ("ewlo2", NT * 5),
                            ("w1r", BI), ("b1r", BI), ("dinv", SL)]:
                sb[name] = p.tile([1, n], F32, tag=name, name=f"sb_{name}")
                nc.sync.dma_start(sb[name][:], frow(name, n))
            # binv in the e = k*100 + p layout used by e_nat
            binv_col = p.tile([100, 3], F32, tag="binv_col")
            nc.sync.dma_start(
                binv_col[:],
                frow("binv", NHE).rearrange("z (k q) -> (z q) k", k=3, q=100))

            # ---- bit-packed masks -> m_sb[r] [JT, NJ, SL] f32 ----
            mraw = p.tile([JT, NJ, 2 * MB], U8, tag="mraw")
            nc.sync.dma_start(
                mraw[:],
                upk[:, UOFF_MASKS:UOFF_MASKS + G * 2 * MB].rearrange(
                    "z (jt q c) -> (z q) jt c", jt=NJ, q=JT, c=2 * MB))
            m_sb = {}
            for r, coef in ((1, ALPHA), (2, BETA)):
                off = (r - 1) * MB
                bits = w.tile([JT, NJ, 8 * MB], U8, tag=f"m{r}bits",
                              name=f"m{r}bits")
                bview = bits[:].rearrange("p t (c k) -> p t c k", k=8)
                for k in range(8):
                    nc.vector.tensor_scalar(
                        bview[:, :, :, k], mraw[:, :, off:off + MB],
                        k, 1, OP.logical_shift_right, OP.bitwise_and)
                t = p.tile([JT, NJ, SL], F32, tag=f"m{r}sb", name=f"m{r}sb")
                nc.vector.tensor_scalar(
                    t[:], bits[:, :, 0:SL],
                    1.0 - coef, coef, OP.mult, OP.add)
                m_sb[r] = t

            # ---- nibble-packed incidence counts -> f32 tiles ----
            def unpack_nibbles(tag, off, kdim, qdim, cdim):
                pk = w.tile([qdim, kdim, cdim], U8, tag=f"{tag}pk",
                            name=f"{tag}pk")
                nc.sync.dma_start(
                    pk[:],
                    upk[:, off:off + kdim * qdim * cdim].rearrange(
                        "z (k q c) -> (z q) k c", k=kdim, q=qdim, c=cdim))
                bits = w.tile([qdim, kdim, 2 * cdim], U8, tag=f"{tag}bits",
                              name=f"{tag}bits")
                bview = bits[:].rearrange("p k (c j) -> p k c j", j=2)
                for j in range(2):
                    nc.vector.tensor_scalar(
                        bview[:, :, :, j], pk[:],
                        4 * j, 15, OP.logical_shift_right, OP.bitwise_and)
                t = p.tile([qdim, kdim, 2 * cdim], F32, tag=tag, name=tag)
                nc.vector.tensor_scalar_mul(
                    t[:].rearrange("p k e -> p (k e)"),
                    bits[:].rearrange("p k e -> p (k e)"), 1.0)
                return t

            mt_sb = unpack_nibbles("mt_sb", UOFF_MT, 2, 75, NHE // 2)
            mn_sb = unpack_nibbles("mn_sb", UOFF_MN, 3, 100, SL // 2)

            ones4 = p.tile([4, 1], F32, tag="ones4")
            nc.vector.memset(ones4[:], 1.0)

            ewlo_bc = {}
            for r, nm in ((1, "ewlo1"), (2, "ewlo2")):
                t = p.tile([JT, NT * 5], F32, tag=f"ewlo{r}bc",
                           name=f"ewlo{r}bc")
                nc.gpsimd.partition_broadcast(t[:], sb[nm][:])
                ewlo_bc[r] = t

            # ---- h = elu(x @ infer_w.T + infer_b), own slice only ----
            hTIc1 = p.tile([5, BI], F32, tag="hTIc1")
            for half in range(2):
                cs = slice(half * 300, half * 300 + 300)
                hp = psA.tile([5, 300], F32, tag="psA_gen", name="hps")
                nc.tensor.matmul(hp[:], sb["wE"][:], sb["xTaIc"][:, cs],
                                 start=True, stop=True)
                _elu(nc, w, hTIc1[:, cs], hp[:], (5, 300))

            def allgather_hN(src44, tag):
                """AllGather own [4, BI] slice -> hN [JT, NT, 5] (full h)."""
                agin = dr.tile([BI, 4], F32)
                agout = dr.tile([NCORES * BI, 4], F32, addr_space="Shared")
                nc.sync.dma_start(agin[:].rearrange("bi f -> f bi"), src44)
                nc.gpsimd.collective_compute(
                    "AllGather", OP.bypass,
                    replica_groups=[list(range(NCORES))],
                    ins=[agin[:].opt()], outs=[agout[:].opt()])
                hN = p.tile([JT, NT, 5], F32, tag=tag, name=tag)
                for cp in range(NCORES):
                    j0 = cp * SL
                    jt0, p0 = j0 // JT, j0 % JT
                    len0 = min(SL, JT - p0)
                    runs = [(jt0, p0, 0, len0)]
                    if len0 < SL:
                        runs.append((jt0 + 1, 0, len0, SL - len0))
                    for (jt, pstart, i0, ln) in runs:
                        dst = hN[pstart:pstart + ln, :, 0:4] \
                            .rearrange("p (b jt) f -> p b jt f", b=B)[:, :, jt, :]
                        sap = agout[cp * BI:(cp + 1) * BI, :] \
                            .rearrange("(b i) f -> i b f", b=B)[i0:i0 + ln, :, :]
                        nc.sync.dma_start(dst, sap)
                nc.vector.memset(hN[:, :, 4:5], 1.0)
                return hN

            hN1 = allgather_hN(hTIc1[0:4, :], "hN1")

            def bn(yT, tag):
                """BatchNorm per gene over (batch, feat); yT [4, BI] sbuf AP.
                Two-pass: mean, subtract, then variance of the residual."""
                srow = w.tile([1, BI], F32, tag="bn_sr", name="bn_sr")
                for half in range(2):
                    cs = slice(half * 300, half * 300 + 300)
                    sp = psA.tile([1, 300], F32, tag="psA_gen", name="bn_sp")
                    nc.tensor.matmul(sp[:], ones4[:], yT[:, cs],
                                     start=True, stop=True)
                    nc.vector.tensor_copy(srow[:, cs], sp[:])
                m = w.tile([1, SL], F32, tag="bn_m", name="bn_m")
                nc.vector.tensor_reduce(
                    m[:], srow[:].rearrange("p (b i) -> p i b", b=B),
                    AX.X, OP.add)
                nc.vector.tensor_scalar_mul(m[:], m[:], 1.0 / 16.0)
                m600 = w.tile([1, BI], F32, tag="bn_m600", name="bn_m600")
                for b in range(B):
                    cs = slice(b * SL, b * SL + SL)
                    nc.vector.tensor_copy(m600[:, cs], m[:])
                mbc = w.tile([4, BI], F32, tag="bn_mbc", name="bn_mbc")
                nc.gpsimd.partition_broadcast(mbc[:], m600[:])
                ybar = w.tile([4, BI], F32, tag="bn_ybar", name="bn_ybar")
                nc.vector.tensor_sub(ybar[:], yT, mbc[:])
                sq = w.tile([4, BI], F32, tag="bn_sq", name="bn_sq")
                nc.vector.tensor_tensor(sq[:], ybar[:], ybar[:], OP.mult)
                qrow = w.tile([1, BI], F32, tag="bn_qr", name="bn_qr")
                for half in range(2):
                    cs = slice(half * 300, half * 300 + 300)
                    qp = psA.tile([1, 300], F32, tag="psA_gen", name="bn_qp")
                    nc.tensor.matmul(qp[:], ones4[:], sq[:, cs],
                                     start=True, stop=True)
                    nc.vector.tensor_copy(qrow[:, cs], qp[:])
                var = w.tile([1, SL], F32, tag="bn_var", name="bn_var")
                nc.vector.tensor_reduce(
                    var[:], qrow[:].rearrange("p (b i) -> p i b", b=B),
                    AX.X, OP.add)
                nc.vector.tensor_scalar(var[:], var[:], 1.0 / 16.0, BN_EPS,
                                        OP.mult, OP.add)
                rec = w.tile([1, SL], F32, tag="bn_rec", name="bn_rec")
                nc.vector.reciprocal(rec[:], var[:])
                rstd = w.tile([1, SL], F32, tag="bn_rstd", name="bn_rstd")
                nc.scalar.activation(rstd[:], rec[:], AF.Sqrt)
                r600 = w.tile([1, BI], F32, tag="bn_r600", name="bn_r600")
                for b in range(B):
                    cs = slice(b * SL, b * SL + SL)
                    nc.vector.tensor_copy(r600[:, cs], rstd[:])
                rbc = w.tile([4, BI], F32, tag="bn_rbc", name="bn_rbc")
                nc.gpsimd.partition_broadcast(rbc[:], r600[:])
                out = p.tile([4, BI], F32, tag=f"{tag}out", name=f"{tag}out")
                nc.vector.tensor_tensor(out[:], ybar[:], rbc[:], OP.mult)
                return out

            def round_(r, hN, hT_ic, ewhi, nwEa, nwEb, mwEa, mwEb):
                """One round. hN [120,40,5]; hT_ic [5,BI] (row 4 ones).
                Returns updT [4, BI]."""
                vrow = w.tile([1, BI], F32, tag="rnd_vrow", name="rnd_vrow")
                for half in range(2):
                    cs = slice(half * 300, half * 300 + 300)
                    vp = psA.tile([1, 300], F32, tag="psA_gen", name="vp")
                    nc.tensor.matmul(vp[:], ewhi, hT_ic[:, cs],
                                     start=True, stop=True)
                    nc.vector.tensor_copy(vrow[:, cs], vp[:])
                vb = p.tile([128, BI], F32, tag="vb", name="vb")
                nc.gpsimd.partition_broadcast(vb[:], vrow[:])
                scr = w.tile([JT, NT * 5], F32, tag="uscr")
                nc.vector.tensor_tensor(
                    scr[:], hN[:].rearrange("p t f -> p (t f)"),
                    ewlo_bc[r][:], OP.mult)
                ucol = w.tile([JT, NT], F32, tag="rnd_ucol", name="rnd_ucol")
                nc.vector.tensor_reduce(
                    ucol[:], scr[:].rearrange("p (t f) -> p t f", f=5),
                    AX.X, OP.add)
                recv1o = w.tile([5, BI], F32, tag="rnd_recv1", name="rnd_recv1")
                nc.vector.memset(recv1o[:, :], 1.0)
                rsrow = w.tile([1, BI], F32, tag="rnd_rs", name="rnd_rs")
                for b in range(B):
                    Wb = w.tile([JT, NJ, SL], F32, tag="Wb", name="Wb", bufs=3)
                    for jt in range(NJ):
                        t = b * NJ + jt
                        nc.scalar.activation(
                            Wb[:, jt, :], vb[0:JT, b * SL:(b + 1) * SL],
                            AF.Sigmoid, bias=ucol[:, t:t + 1])
                    eng = nc.vector if b % 2 == 0 else nc.gpsimd
                    eng.tensor_tensor(
                        Wb[:].rearrange("p t i -> p (t i)"),
                        Wb[:].rearrange("p t i -> p (t i)"),
                        m_sb[r][:].rearrange("p t i -> p (t i)"), OP.mult)
                    rp = psA.tile([5, SL], F32, tag="recvps", name="rp", bufs=2)
                    for jt in range(NJ):
                        t = b * NJ + jt
                        nc.tensor.matmul(rp[:], hN[:, t, :], Wb[:, jt, :],
                                         start=(jt == 0), stop=(jt == NJ - 1))
                    cs = slice(b * SL, (b + 1) * SL)
                    nc.vector.tensor_copy(recv1o[0:4, cs], rp[0:4, :])
                    # rs row: DMA (not a compute op) — partition-offset APs are
                    # only broken on compute engines
                    rv5 = w.tile([5, SL], F32, tag="rv5", name="rv5", bufs=2)
                    nc.vector.tensor_copy(rv5[:], rp[:])
                    nc.sync.dma_start(rsrow[:, cs], rv5[4:5, :])
                rsbc = w.tile([4, BI], F32, tag="rnd_rsbc", name="rnd_rsbc")
                nc.gpsimd.partition_broadcast(rsbc[:], rsrow[:])
                recv2 = w.tile([4, BI], F32, tag="rnd_recv2", name="rnd_recv2")
                nc.vector.tensor_tensor(recv2[:], hT_ic[0:4, :], rsbc[:],
                                        OP.mult)
                # A = elu(nwA @ [recv1;1] + nwB @ recv2); Acat row 4 stays ones
                Acat = w.tile([5, BI], F32, tag="rnd_Acat", name="rnd_Acat")
                nc.vector.memset(Acat[:, :], 1.0)
                for half in range(2):
                    cs = slice(half * 300, half * 300 + 300)
                    ap = psA.tile([4, 300], F32, tag="psA_gen", name="ap")
                    nc.tensor.matmul(ap[:], nwEa, recv1o[:, cs],
                                     start=True, stop=False)
                    nc.tensor.matmul(ap[:], nwEb, recv2[:, cs],
                                     start=False, stop=True)
                    _elu(nc, w, Acat[0:4, cs], ap[:], (4, 300))
                updT = p.tile([4, BI], F32, tag=f"r{r}upd")
                for half in range(2):
                    cs = slice(half * 300, half * 300 + 300)
                    up = psA.tile([4, 300], F32, tag="psA_gen", name="up")
                    nc.tensor.matmul(up[:], mwEa, Acat[:, cs],
                                     start=True, stop=False)
                    nc.tensor.matmul(up[:], mwEb, hT_ic[0:4, cs],
                                     start=False, stop=True)
                    _elu(nc, w, updT[:, cs], up[:], (4, 300))
                return updT

            # ================= round 1 =================
            upd1 = round_(1, hN1, hTIc1[:], sb["ewhi1"][:], sb["nwE1a"][:],
                          sb["nwE1b"][:], sb["mwE1a"][:], sb["mwE1b"][:])
            # h2 = elu(upd1 * diag(W1) + b1), then BN
            w1bc = w.tile([4, BI], F32, tag="w1bc")
            b1bc = w.tile([4, BI], F32, tag="b1bc")
            nc.gpsimd.partition_broadcast(w1bc[:], sb["w1r"][:])
            nc.gpsimd.partition_broadcast(b1bc[:], sb["b1r"][:])
            h2pre = w.tile([4, BI], F32, tag="h2pre")
            nc.vector.tensor_tensor(h2pre[:], upd1[:], w1bc[:], OP.mult)
            nc.vector.tensor_add(h2pre[:], h2pre[:], b1bc[:])
            h2T = w.tile([4, BI], F32, tag="h2T")
            _elu(nc, w, h2T[:], h2pre[:], (4, BI))
            h2bn = bn(h2T[:], "bn1")

            # ---- AllGather #2: h2bn slices -> full h in hN2 layout ----
            hN2 = allgather_hN(h2bn[:], "hN2")
            hTIc2 = p.tile([5, BI], F32, tag="hTIc2")
            nc.vector.memset(hTIc2[:, :], 1.0)
            nc.vector.tensor_copy(hTIc2[0:4, :], h2bn[:])

            # ================= round 2 =================
            upd2 = round_(2, hN2, hTIc2[:], sb["ewhi2"][:], sb["nwE2a"][:],
                          sb["nwE2b"][:], sb["mwE2a"][:], sb["mwE2b"][:])
            upd2bn = bn(upd2[:], "bn2")

            # ---- hypergraph partial: E_part = M[:,Ic] @ (sum_b upd2bn @ hg_w.T)
            s0T = w.tile([4, SL], F32, tag="s0T")
            nc.vector.tensor_reduce(
                s0T[:], upd2bn[:].rearrange("p (b i) -> p i b", b=B),
                AX.X, OP.add)
            s1p = psA.tile([4, SL], F32, tag="psA_gen", name="s1p")
            nc.tensor.matmul(s1p[:], sb["hgwT"][:], s0T[:], start=True,
                             stop=True)
            s1sb = w.tile([4, SL], F32, tag="s1sb")
            nc.vector.tensor_copy(s1sb[:], s1p[:])
            s1d = dr.tile([SL, 4], F32)
            nc.sync.dma_start(s1d[:].rearrange("i f -> f i"), s1sb[:])
            s1n = p.tile([75, 2, 4], F32, tag="s1n")
            nc.sync.dma_start(
                s1n[:], s1d[:].rearrange("(k q) f -> q k f", q=75))
            ep = psA.tile([4, NHE], F32, tag="psA_gen", name="ep")
            for k in range(2):
                nc.tensor.matmul(ep[:], s1n[:, k, :], mt_sb[:, k, :],
                                 start=(k == 0), stop=(k == 1))

            # ---- AllReduce: E = sum over cores of E_part (natural [NHE,4]) ----
            epsb = w.tile([4, NHE], F32, tag="epsb")
            nc.vector.tensor_copy(epsb[:], ep[:])
            arin = dr.tile([NHE, 4], F32)
            arout = dr.tile([NHE, 4], F32, addr_space="Shared")
            nc.sync.dma_start(arin[:].rearrange("e f -> f e"), epsb[:])
            nc.gpsimd.collective_compute(
                "AllReduce", OP.add,
                replica_groups=[list(range(NCORES))],
                ins=[arin[:].opt()], outs=[arout[:].opt()])
            e_nat = p.tile([100, 3, 4], F32, tag="e_nat")
            nc.sync.dma_start(
                e_nat[:], arout[:].rearrange("(k q) f -> q k f", q=100))
            # scale hyperedge features by binv (e = k*100 + p layout)
            for k in range(3):
                nc.vector.tensor_scalar_mul(
                    e_nat[:, k, :], e_nat[:, k, :], binv_col[:, k:k + 1])
            hxp = psA.tile([4, SL], F32, tag="psA_gen", name="hxp")
            for k in range(3):
                nc.tensor.matmul(hxp[:], e_nat[:, k, :], mn_sb[:, k, :],
                                 start=(k == 0), stop=(k == 2))
            # scale node features by dinv (free-dim vector), add bias, elu
            dbc = w.tile([4, SL], F32, tag="dbc")
            nc.gpsimd.partition_broadcast(dbc[:], sb["dinv"][:])
            hxs = w.tile([4, SL], F32, tag="hxs")
            nc.vector.tensor_tensor(hxs[:], hxp[:], dbc[:], OP.mult)
            hxpre = w.tile([4, SL], F32, tag="hxpre")
            nc.vector.tensor_scalar_add(hxpre[:], hxs[:], sb["hgb"][:])
            hxT = w.tile([4, SL], F32, tag="hxT")
            _elu(nc, w, hxT[:], hxpre[:], (4, SL))

            # ---- final: out = elu(mm3A @ [upd2bn;1] + mm3B @ hx + b) ----
            u2cat = w.tile([5, BI], F32, tag="u2cat")
            nc.vector.memset(u2cat[:, :], 1.0)
            nc.vector.tensor_copy(u2cat[0:4, :], upd2bn[:])
            hx600 = w.tile([4, BI], F32, tag="hx600")
            for b in range(B):
                cs = slice(b * SL, (b + 1) * SL)
                nc.vector.tensor_copy(hx600[:, cs], hxT[:])
            outT = w.tile([4, BI], F32, tag="outTsb")
            for half in range(2):
                cs = slice(half * 300, half * 300 + 300)
                op_ = psA.tile([4, 300], F32, tag="psA_gen", name="op_")
                nc.tensor.matmul(op_[:], sb["mm3Ea"][:], u2cat[:, cs],
                                 start=True, stop=False)
                nc.tensor.matmul(op_[:], sb["mm3Eb"][:], hx600[:, cs],
                                 start=False, stop=True)
                _elu(nc, w, outT[:, cs], op_[:], (4, 300))
            nc.sync.dma_start(out_d[:], outT[:])

    nc.compile()
    return nc


def _prep_inputs(x, edge1, edge2, W1, b1, infer_w, infer_b, mlp_e1_w, mlp_e1_b,
                 mlp_e2_w, mlp_e2_b, nodes1_w, nodes1_b, nodes2_w, nodes2_b,
                 mm1_w, mm1_b, mm2_w, mm2_b, mm3_w, mm3_b, hg_w, hg_b,
                 hyper_nodes, hyper_edges):
    f = np.float32
    xT = np.ascontiguousarray(x.transpose(0, 2, 1).astype(f))  # [B, NIN, G]
    xTa = np.concatenate([xT.transpose(1, 0, 2).reshape(NIN, B * G),
                          np.ones((1, B * G), f)], axis=0)
    wE = np.zeros((NIN + 1, 5), f)
    wE[:NIN, :4] = infer_w.T
    wE[NIN, :4] = infer_b
    wE[NIN, 4] = 1.0

    def split5(wgt, bias):
        a = np.zeros((5, 4), f)
        a[:4] = wgt[:, :4].T
        a[4] = bias
        b_ = np.ascontiguousarray(wgt[:, 4:].T.astype(f))
        return a, b_

    nwE1a, nwE1b = split5(nodes1_w, nodes1_b)
    mwE1a, mwE1b = split5(mm1_w, mm1_b)
    nwE2a, nwE2b = split5(nodes2_w, nodes2_b)
    mwE2a, mwE2b = split5(mm2_w, mm2_b)
    mm3Ea, mm3Eb = split5(mm3_w, mm3_b)

    def ewparts(ew, eb):
        lo5 = np.zeros(5, f)
        lo5[:4] = ew[0, :4]
        lor = np.tile(lo5, NT)                          # [200]
        hi = np.zeros(5, f)
        hi[:4] = ew[0, 4:8]
        hi[4] = eb[0]
        return lor.astype(f), hi
    ewlo1, ewhi1 = ewparts(mlp_e1_w, mlp_e1_b)
    ewlo2, ewhi2 = ewparts(mlp_e2_w, mlp_e2_b)

    # bit-packed masks: byte c bit k of row j <- edge[j, Ic][8c+k]
    m1 = edge1.T.astype(np.uint8)                       # [G(j), G(i)]
    m2 = edge2.T.astype(np.uint8)

    M = np.zeros((NHE, G), f)
    np.add.at(M, (hyper_edges, hyper_nodes), 1.0)
    deg = M.sum(0)
    dinv = np.where(deg > 0, 1.0 / np.maximum(deg, 1), 0.0).astype(f)
    bdeg = B * M.sum(1)
    binv = np.where(bdeg > 0, 1.0 / np.maximum(bdeg, 1), 0.0).astype(f)
    Mu8 = M.astype(np.uint8)

    w1d = np.diag(W1).astype(f)
    hgwT = hg_w.T.astype(f)
    hgb = hg_b.astype(f)

    in_maps = []
    for c in range(NCORES):
        Ic = slice(c * SL, (c + 1) * SL)
        xTaIc = np.concatenate([xTa[:, b * G + c * SL: b * G + (c + 1) * SL]
                                for b in range(B)], axis=1)
        fpk = np.zeros(FPK_N, f)
        for nm, arr in [
            ("xTaIc", xTaIc), ("wE", wE), ("ewhi1", ewhi1), ("ewhi2", ewhi2),
            ("nwE1a", nwE1a), ("nwE1b", nwE1b), ("mwE1a", mwE1a),
            ("mwE1b", mwE1b), ("nwE2a", nwE2a), ("nwE2b", nwE2b),
            ("mwE2a", mwE2a), ("mwE2b", mwE2b), ("mm3Ea", mm3Ea),
            ("mm3Eb", mm3Eb), ("hgwT", hgwT), ("hgb", hgb),
            ("ewlo1", ewlo1), ("ewlo2", ewlo2),
            ("w1r", np.tile(w1d[Ic], B)), ("b1r", np.tile(b1.astype(f)[Ic], B)),
            ("binv", binv), ("dinv", dinv[Ic]),
        ]:
            av = np.asarray(arr, f).ravel()
            fpk[_FOFF[nm]:_FOFF[nm] + av.size] = av
        upkb = np.zeros(UPK_N, np.uint8)
        for r, msk in ((0, m1), (1, m2)):
            pk = np.packbits(msk[:, Ic], axis=1, bitorder="little")  # [G, 19]
            dst = upkb[UOFF_MASKS:UOFF_MASKS + G * 2 * MB].reshape(G, 2 * MB)
            dst[:, r * MB:r * MB + pk.shape[1]] = pk
        mt = np.ascontiguousarray(Mu8[:, Ic].T)                # [SL, NHE]
        mn = np.ascontiguousarray(Mu8[:, Ic])                  # [NHE, SL]
        upkb[UOFF_MT:UOFF_MT + SL * NHE // 2] = \
            (mt[:, 0::2] | (mt[:, 1::2] << 4)).ravel()
        upkb[UOFF_MN:UOFF_MN + NHE * SL // 2] = \
            (mn[:, 0::2] | (mn[:, 1::2] << 4)).ravel()
        in_maps.append({"fpk": fpk[None, :], "upk": upkb[None, :]})
    return in_maps


def kernel(**inputs):
    inputs = {k: np.asarray(v) for k, v in inputs.items()}
    if "nc" not in _COMPILED:
        _COMPILED["nc"] = _build()
    nc = _COMPILED["nc"]
    in_maps = _prep_inputs(**inputs)
    # the axon-tunneled device occasionally reports a transient
    # NRT_EXEC_UNIT_UNRECOVERABLE; it clears after a short wait
    last = None
    for attempt in range(4):
        try:
            res = bass_utils.run_bass_kernel_spmd(
                nc, in_maps, core_ids=list(range(NCORES)))
            break
        except Exception as e:
            last = e
            import time as _time
            _time.sleep(5 * (attempt + 1))
    else:
        raise last
    out = np.empty((B, G, H), np.float32)
    for c in range(NCORES):
        oT = res.results[c]["outT"].reshape(4, B, SL)   # [f, b, i]
        out[:, c * SL:(c + 1) * SL, :] = oT.transpose(1, 2, 0)
    return out
